# revision 22
# baseline (speedup 1.0000x reference)
"""GNN message-passing kernel for Trainium2 (8 NeuronCores, data-parallel).

Computes msg = vs @ W + b.sum(0) for vs [2M, 8] f32, W/b [8, 64] f32.

Strategy (v3 — DMA-traffic minimized, zero mid-stream DMA gaps):
  - Shard vs rows 8 ways (250k rows/core); W/b replicated.
  - Precision: the harness gate is rel_err < 2e-2; single fp16 inputs and
    fp16 outputs give ~3e-4, so skip the hi/lo split entirely.
      * input:  8 f16 values/node + one shared ones row = ~16.2 B/node
        -> 4.05 MB/core
      * output: f16 [250k, 64] = 32 MB/core, upcast to f32 on the host
        (plus exact host-side correction of the f16-rounded bias).
    Total 36.25 MB/core vs 76 MB for the old fp32-out/hi-lo kernel
    -> 100.7 us of DMA at the 360 GB/s model bandwidth.
  - Layout: host packs the input pre-transposed into the matmul stationary
    layout, so there are no on-device PE transposes. A chunk is 1792 nodes =
    128 partitions x 14 t-blocks; lhsT[8t+i, m] = vs[c*1792 + m*14 + t, i],
    row 112 is a constant 1. The block-diagonal ws [113, 896] has
    ws[8t+i, 64t+h] = W[i,h] and ws[112, :] = tile(bsum, 14) (one ones row
    serves every t-block), so out[m, 64t+h] = msg[node(m,t), h] with fully
    contiguous output DMA (1792 B per partition per chunk).
  - The whole input (35.8 KB/partition) stays SBUF-resident, loaded up
    front by 10 double-supertile DMAs whose 1.25 us transfers hide the
    ~650 ns per-DMA issue cadence (no inter-transfer gaps).
  - Two matmuls per chunk (N=448 each, one per PSUM bank); PSUM f32 is
    evacuated to f16 SBUF by plain copies alternating between the DVE and
    ACT engines (~64/75 us per engine, under the DMA roofline).
  - Output in supertiles of 7 chunks via gpsimd/SWDGE DMAs so their waits
    never block the SP input queue; 139 full chunks + one 912-node tail
    chunk [114 x 8] with disjoint DRAM rows (no WAW serialization).
  - Cost-model timeline: 104.4 us = 2.0 us startup latency + 100.7 us DMA
    (96.4% occupancy, gapless) + 1.7 us drain. Baseline was 228 us.
"""

import numpy as np
import concourse.bacc as bacc
import concourse.mybir as mybir
from concourse.tile import TileContext
from concourse.bass_utils import run_bass_kernel_spmd

F32 = mybir.dt.float32
F16 = mybir.dt.float16

B = 2_000_000
NCORES = 8
NS = B // NCORES          # 250_000 nodes per core
TB = 14                   # t-blocks per chunk
KROWS = 8 * TB + 1        # 113 contraction rows (112 data + 1 shared ones row)
CHUNK = 128 * TB          # 1792 nodes per chunk
NFULL = NS // CHUNK       # 139 full chunks
NCH = NFULL + 1           # +1 tail chunk (912 nodes, disjoint rows)
G = 7                     # chunks per supertile
NST = NCH // G            # 20 supertiles
NCOL = 64 * TB            # 896 ws columns / out f16 elems per chunk
# K-row layout: data rows at 8t+i (partitions 0..111), plus ONE shared ones
# row at 112 whose ws row carries bsum for every t-block (its ws row spans
# all blocks' columns, so per-block ones rows are unnecessary).
KDATA = 8 * TB            # 112 data rows
IN_DMAS = NST // 2        # input loaded as 10 double-supertile DMAs
# Tail chunk: the last 912 nodes as [M=114 partitions, T=8 t-blocks]. Its
# data rows (0..64) and ws columns (0..512) are the top-left block of the
# regular layout, so it shares ws and the full-K matmul (rows 64..112 are
# host-zeroed for the tail columns). Disjoint DRAM rows -> no WAW stall on
# the final output DMA.
TM, TT = 114, 8
TAILN = TM * TT           # 912
TAILCOL = 64 * TT         # 512
PRE = NST                 # whole input fits in SBUF (35.8 KB/partition):
                          # prefetch everything so the DMA engines never
                          # starve waiting on compute mid-stream.

_nc_cache = None


def _build():
    nc = bacc.Bacc()
    pin = nc.dram_tensor("pin", [KROWS, NCH * 128], F16, kind="ExternalInput")
    ws = nc.dram_tensor("ws", [KROWS, NCOL], F16, kind="ExternalInput")
    out = nc.dram_tensor("out", [NS, 64], F16, kind="ExternalOutput")

    with TileContext(nc) as tc:
        with (
            tc.tile_pool(name="const", bufs=1) as cpool,
            tc.tile_pool(name="outp", bufs=3) as out_pool,
            tc.tile_pool(name="pmm", bufs=4, space="PSUM") as pmm_pool,
        ):
            ws_sb = cpool.tile([128, NCOL], F16)
            # The whole per-core input is only 35.8 KB/partition: keep it
            # SBUF-resident in one tile, loaded by IN_DMAS double-supertile
            # DMAs (1254 ns transfers) so the 625 ns HWDGE/SEQ issue cadence
            # never gaps the DMA engines.
            mega = cpool.tile([128, NCH * 128], F16)
            in_cols = NCH * 128 // IN_DMAS

            def issue_in(p):
                nc.sync.dma_start(
                    out=mega[:KROWS, p * in_cols : (p + 1) * in_cols],
                    in_=pin[:, p * in_cols : (p + 1) * in_cols],
                )

            issue_in(0)
            # ws after the first input DMA so the pipeline's first transfer
            # isn't delayed behind it.
            nc.sync.dma_start(out=ws_sb[:KROWS, :], in_=ws[:, :])
            for p in range(1, IN_DMAS):
                issue_in(p)

            for s in range(NST):
                out_sb = out_pool.tile([128, G * NCOL], F16, tag="out")
                nreg = G if s < NST - 1 else G - 1
                for j in range(nreg):
                    c = s * G + j
                    lhsT = mega[:KROWS, c * 128 : (c + 1) * 128]
                    mm = pmm_pool.tile([128, 1024], F32, tag="mm")
                    nc.tensor.matmul(
                        mm[:, 0:448], lhsT, ws_sb[:KROWS, 0:448],
                        start=True, stop=True,
                    )
                    nc.tensor.matmul(
                        mm[:, 512:960], lhsT, ws_sb[:KROWS, 448:896],
                        start=True, stop=True,
                    )
                    src = mm[:].rearrange("p (k n) -> p k n", k=2)[:, :, :448]
                    dst = out_sb[:, j * NCOL : (j + 1) * NCOL].rearrange(
                        "p (k n) -> p k n", k=2
                    )
                    # Alternate evacuation between DVE and ACT (4:3 toward the
                    # cheaper-per-chunk ACT) so neither engine bottlenecks.
                    if j % 2 == 0 and j < 6:
                        nc.vector.tensor_copy(out=dst, in_=src)
                    else:
                        nc.scalar.copy(out=dst, in_=src)
                if s < NST - 1:
                    base = s * G * CHUNK
                    out_ap = out[base : base + G * CHUNK, :].rearrange(
                        "(c m t) h -> m c (t h)", c=G, m=128, t=TB
                    )
                    src_ap = out_sb[:, :].rearrange("p (c n) -> p c n", c=G)
                    nc.gpsimd.dma_start(out=out_ap, in_=src_ap)
                else:
                    # Last supertile: 6 regular chunks + the tail chunk.
                    lhsT = mega[:KROWS, NFULL * 128 : NFULL * 128 + TM]
                    mm = pmm_pool.tile([128, 1024], F32, tag="mm")
                    nc.tensor.matmul(
                        mm[:TM, 0:TAILCOL], lhsT, ws_sb[:KROWS, 0:TAILCOL],
                        start=True, stop=True,
                    )
                    nc.vector.tensor_copy(
                        out=out_sb[:TM, 6 * NCOL : 6 * NCOL + TAILCOL],
                        in_=mm[:TM, 0:TAILCOL],
                    )
                    base = s * G * CHUNK
                    out_ap = out[base : base + 6 * CHUNK, :].rearrange(
                        "(c m t) h -> m c (t h)", c=6, m=128, t=TB
                    )
                    src_ap = out_sb[:, : 6 * NCOL].rearrange(
                        "p (c n) -> p c n", c=6
                    )
                    nc.gpsimd.dma_start(out=out_ap, in_=src_ap)
                    tail_ap = out[NS - TAILN : NS, :].rearrange(
                        "(m t) h -> m (t h)", m=TM, t=TT
                    )
                    nc.gpsimd.dma_start(
                        out=tail_ap,
                        in_=out_sb[:TM, 6 * NCOL : 6 * NCOL + TAILCOL],
                    )
    nc.compile()
    return nc


def _get_nc():
    global _nc_cache
    if _nc_cache is None:
        _nc_cache = _build()
    return _nc_cache


def _pack_core(v16: np.ndarray) -> np.ndarray:
    """[NS, 8] f16 -> [113, NCH*128] stationary-layout f16: data rows at
    8t+i (0..112), one shared constant ones row at 112."""
    pin = np.zeros((KROWS, NCH * 128), dtype=np.float16)
    # [c, m, t, i] -> [t, i, c, m] -> [8*TB, NFULL*128]
    pin[:KDATA, : NFULL * 128] = (
        v16[: NFULL * CHUNK]
        .reshape(NFULL, 128, TB, 8)
        .transpose(2, 3, 0, 1)
        .reshape(KDATA, NFULL * 128)
    )
    pin[: 8 * TT, NFULL * 128 : NFULL * 128 + TM] = (
        v16[NFULL * CHUNK :].reshape(TM, TT, 8).transpose(1, 2, 0).reshape(8 * TT, TM)
    )
    pin[KDATA, : NFULL * 128] = np.float16(1.0)
    pin[KDATA, NFULL * 128 : NFULL * 128 + TM] = np.float16(1.0)
    return pin


def kernel(vs: np.ndarray, W: np.ndarray, b: np.ndarray, _trace=False):
    vs = np.asarray(vs, dtype=np.float32)
    W = np.asarray(W, dtype=np.float32)
    b = np.asarray(b, dtype=np.float32)

    nc = _get_nc()

    W16 = W.astype(np.float16)
    bsum = b.sum(axis=0, dtype=np.float32)
    bsum16 = bsum.astype(np.float16)
    resid = bsum - bsum16.astype(np.float32)   # exact bias correction (host)

    ws = np.zeros((KROWS, NCOL), dtype=np.float16)
    for t in range(TB):
        ws[8 * t : 8 * t + 8, 64 * t : 64 * t + 64] = W16
    ws[KDATA, :] = np.tile(bsum16, TB)

    vs16 = vs.reshape(B, 8).astype(np.float16)
    in_maps = [
        {"pin": _pack_core(vs16[k * NS : (k + 1) * NS]), "ws": ws}
        for k in range(NCORES)
    ]

    res = run_bass_kernel_spmd(nc, in_maps, core_ids=list(range(NCORES)))
    out = np.concatenate([r["out"] for r in res.results], axis=0)
    out = out.astype(np.float32)
    out += resid
    if _trace:
        kernel.last_result = res
    return out
